# revision 8
# baseline (speedup 1.0000x reference)
"""Luong concat attention, data-parallel over batch across 8 TRN2 NeuronCores.

Reference computation (per batch row b):
    q_proj  = q[b] @ Wq.T                    # [DA]
    e_proj  = e[b] @ We.T                    # [S, DA]
    energies = tanh(q_proj + e_proj)         # [S, DA]
    scores  = energies @ V.T  (+ mask)       # [S]
    probs   = softmax(scores)                # [S]
    ctx     = probs @ e[b]                   # [DE]

Sharding: B=32 split 4 per core; no collectives. The dominant matmul
(e_proj, 137 GFLOP total) runs in bf16 with fp32 PSUM accumulation.

Per-core layout strategy:
  - e arrives in HBM as bf16 [4, S, DE] (host casts).
  - For e_proj the contraction is over DE, so e is DMA-transposed
    (hardware xbar) into SBUF as eT chunks [128(de), S].
  - e_proj computed as out[da_tile(128 part), tok(512 free)] so tanh's
    q_proj addend is a per-partition ACT bias, and scores = V . energies
    is a PE matmul contracting the partition dim.
  - softmax on a [1, S] row (mask pre-folded in as an additive -1e9 row
    via a K=1 matmul into the scores PSUM accumulation).
  - probs go to HBM fp32, and are gathered back transposed+cast to bf16
    [128(s), 16] via a SWDGE (gpsimd) casting DMA, which is the lhsT for
    the context matmul against natural-layout e rows.
"""

import numpy as np
import ml_dtypes

# Problem constants (full problem; per-core shard sizes derived).
B, S, DD, DE, DA = 32, 2048, 1024, 1024, 1024
N_CORES = 8
BL = B // N_CORES  # local batches per core
NEG = -1000000000.0

_CACHE = {}


def build_kernel_nc(BL=BL, S=S, DD=DD, DE=DE, DA=DA):
    """Build + compile the per-core Bass program. Returns nc."""
    import concourse.bacc as bacc
    import concourse.mybir as mybir
    import concourse.tile as tile

    f32 = mybir.dt.float32
    bf16 = mybir.dt.bfloat16

    KD = DD // 128  # dd chunks (q_proj contraction)
    KE = DE // 128  # de chunks (e_proj contraction)
    MA = DA // 128  # da tiles
    TK = 512        # token tile (free dim per matmul)
    NT = S // TK    # token tiles per batch
    SC = S // 128   # s chunks (context contraction)
    CW = min(512, DE)  # context output tile width
    DH = DE // CW   # context output tiles

    nc = bacc.Bacc("TRN2", target_bir_lowering=False, debug=False,
                   num_devices=N_CORES)

    # I/O (per-core shapes)
    e_t = nc.dram_tensor("e", [BL, S, DE], bf16, kind="ExternalInput")
    wet_t = nc.dram_tensor("wet", [DE, DA], bf16, kind="ExternalInput")
    wqt_t = nc.dram_tensor("wqt", [DD, DA], f32, kind="ExternalInput")
    qt_t = nc.dram_tensor("qt", [DD, BL], f32, kind="ExternalInput")
    vt_t = nc.dram_tensor("vt", [128, DA // 128], bf16, kind="ExternalInput")
    mask_t = nc.dram_tensor("maskadd", [1, BL * S], bf16, kind="ExternalInput")
    ctx_t = nc.dram_tensor("ctx", [BL, DE], f32, kind="ExternalOutput")
    attn_t = nc.dram_tensor("attn", [BL, S], f32, kind="ExternalOutput")

    e = e_t.ap()
    ctx_o = ctx_t.ap()
    attn_o = attn_t.ap()

    Tanh = mybir.ActivationFunctionType.Tanh
    Exp = mybir.ActivationFunctionType.Exp
    X = mybir.AxisListType.X

    with tile.TileContext(nc) as tc:
        from contextlib import ExitStack
        with ExitStack() as ctx:
            const_pool = ctx.enter_context(tc.tile_pool(name="const", bufs=1))

            # ---- resident constants -------------------------------------
            wet_sb = const_pool.tile([128, KE, DA], bf16, tag="wet")
            nc.sync.dma_start(
                out=wet_sb, in_=wet_t.ap().rearrange("(k p) a -> p k a", p=128))
            vt_sb = const_pool.tile([128, DA // 128], bf16, tag="vt")
            nc.sync.dma_start(out=vt_sb, in_=vt_t.ap())
            mask_sb = const_pool.tile([1, BL * S], bf16, tag="mask")
            nc.sync.dma_start(out=mask_sb, in_=mask_t.ap())
            qt_sb = const_pool.tile([128, KD, BL], f32, tag="qt")
            nc.sync.dma_start(
                out=qt_sb, in_=qt_t.ap().rearrange("(k p) b -> p k b", p=128))
            ones_sb = const_pool.tile([1, 1], bf16, tag="ones")
            nc.vector.memset(ones_sb, 1.0)
            qproj_sb = const_pool.tile([128, MA, BL], f32, tag="qproj")

            # ---- main pools ---------------------------------------------
            et_pool = ctx.enter_context(tc.tile_pool(name="et", bufs=2))
            en_pool = ctx.enter_context(tc.tile_pool(name="en", bufs=2))
            er_pool = ctx.enter_context(tc.tile_pool(name="er", bufs=3))
            sm_pool = ctx.enter_context(tc.tile_pool(name="sm", bufs=2))
            rd_pool = ctx.enter_context(tc.tile_pool(name="rd", bufs=4))
            pt_pool = ctx.enter_context(tc.tile_pool(name="pt", bufs=2))
            eps_pool = ctx.enter_context(
                tc.tile_pool(name="eps", bufs=3, space="PSUM"))
            sc_pool = ctx.enter_context(
                tc.tile_pool(name="scp", bufs=2, space="PSUM"))
            cx_pool = ctx.enter_context(
                tc.tile_pool(name="cxp", bufs=1, space="PSUM"))
            dr_pool = ctx.enter_context(
                tc.tile_pool(name="dr", bufs=2, space="DRAM"))

            # ---- phase 1: q_proj in fp32 (tiny, precision matters) ------
            with tc.tile_pool(name="qp", bufs=1) as wq_pool, \
                 tc.tile_pool(name="qpp", bufs=1, space="PSUM") as qp_psum:
                wq = []
                for k in range(KD):
                    w = wq_pool.tile([128, DA], f32, tag=f"wq{k}")
                    nc.sync.dma_start(
                        out=w, in_=wqt_t.ap()[k * 128:(k + 1) * 128, :])
                    wq.append(w)
                for m in range(MA):
                    ps = qp_psum.tile([128, BL], f32, tag="qps")
                    for k in range(KD):
                        nc.tensor.matmul(
                            ps, lhsT=wq[k][:, m * 128:(m + 1) * 128],
                            rhs=qt_sb[:, k, :],
                            start=(k == 0), stop=(k == KD - 1))
                    nc.scalar.copy(qproj_sb[:, m, :], ps)

            for b in range(BL):
                # transposed e chunks for this batch: eT[k] = [128(de), S]
                eT = []
                for k in range(KE):
                    t_ = et_pool.tile([128, S], bf16, tag=f"et{k}")
                    nc.sync.dma_start(
                        out=t_, in_=e[b, :, k * 128:(k + 1) * 128],
                        transpose=True)
                    eT.append(t_)

                scores_sb = sm_pool.tile([1, S], f32, tag="scores")
                for t in range(NT):
                    en_tiles = []
                    for m in range(MA):
                        ps = eps_pool.tile([128, TK], f32, tag="eps")
                        for k in range(KE):
                            nc.tensor.matmul(
                                ps,
                                lhsT=wet_sb[:, k, m * 128:(m + 1) * 128],
                                rhs=eT[k][:, t * TK:(t + 1) * TK],
                                start=(k == 0), stop=(k == KE - 1))
                        en = en_pool.tile([128, TK], bf16, tag=f"en{m}")
                        nc.scalar.activation(
                            en, ps, Tanh, bias=qproj_sb[:, m, b:b + 1])
                        en_tiles.append(en)
                    sps = sc_pool.tile([1, TK], f32, tag="sps")
                    # mask add as K=1 matmul starting the accumulation
                    nc.tensor.matmul(
                        sps, lhsT=ones_sb,
                        rhs=mask_sb[0:1, b * S + t * TK: b * S + (t + 1) * TK],
                        start=True, stop=False)
                    for m in range(MA):
                        nc.tensor.matmul(
                            sps, lhsT=vt_sb[:, m:m + 1], rhs=en_tiles[m],
                            start=False, stop=(m == MA - 1))
                    nc.scalar.copy(scores_sb[0:1, t * TK:(t + 1) * TK], sps)

                # softmax over [1, S]
                negmax = rd_pool.tile([1, 1], f32, tag="negmax")
                nc.vector.reduce_max(negmax, scores_sb, axis=X, negate=True)
                sumexp = rd_pool.tile([1, 1], f32, tag="sumexp")
                pexp = sm_pool.tile([1, S], f32, tag="pexp")
                nc.scalar.activation(pexp, scores_sb, Exp, bias=negmax,
                                     accum_out=sumexp)
                rec = rd_pool.tile([1, 1], f32, tag="rec")
                nc.vector.reciprocal(rec, sumexp)
                probs = sm_pool.tile([1, S], f32, tag="probs")
                nc.vector.tensor_scalar_mul(probs, pexp, rec)
                nc.sync.dma_start(out=attn_o[b:b + 1, :], in_=probs)
                pscr = dr_pool.tile([1, S], f32, tag="pscr")
                nc.sync.dma_start(out=pscr, in_=probs)

                # transposed + bf16-cast probs: [128(s), SC]
                pT = pt_pool.tile([128, SC], bf16, tag="pt")
                nc.gpsimd.dma_start(
                    out=pT, in_=pscr.rearrange("a (c p) -> p (a c)", p=128))

                # context: ctx[b] = probs @ e[b]
                cps = [cx_pool.tile([1, CW], f32, tag=f"cps{h}",
                                    name=f"cps{h}_{b}")
                       for h in range(DH)]
                for c in range(SC):
                    er = er_pool.tile([128, DE], bf16, tag="er")
                    nc.sync.dma_start(
                        out=er, in_=e[b, c * 128:(c + 1) * 128, :])
                    for h in range(DH):
                        nc.tensor.matmul(
                            cps[h], lhsT=pT[:, c:c + 1],
                            rhs=er[:, h * CW:(h + 1) * CW],
                            start=(c == 0), stop=(c == SC - 1))
                for h in range(DH):
                    cs = sm_pool.tile([1, CW], f32, tag=f"ctx{h}")
                    nc.scalar.copy(cs, cps[h])
                    nc.sync.dma_start(
                        out=ctx_o[b:b + 1, h * CW:(h + 1) * CW], in_=cs)

    nc.compile()
    return nc


def shard_inputs(query_state, encoder_hidden_states, attention_mask, W_c, V,
                 n_cores=N_CORES):
    """Host-side prep: slice per-core shards + layout/dtype transforms."""
    bf16 = ml_dtypes.bfloat16
    bl = query_state.shape[0] // n_cores
    dd = query_state.shape[1]
    wet = np.ascontiguousarray(W_c[:, dd:].T).astype(bf16)      # [DE, DA]
    wqt = np.ascontiguousarray(W_c[:, :dd].T).astype(np.float32)  # [DD, DA]
    vt = np.ascontiguousarray(V[0].reshape(-1, 128).T).astype(bf16)  # [128, DA/128]
    in_maps = []
    for c in range(n_cores):
        sl = slice(c * bl, (c + 1) * bl)
        e_bf = encoder_hidden_states[sl].astype(bf16)
        qt = np.ascontiguousarray(query_state[sl].T).astype(np.float32)
        maskadd = np.where(attention_mask[sl] == 0, np.float32(NEG),
                           np.float32(0.0)).astype(bf16).reshape(1, -1)
        in_maps.append({
            "e": e_bf, "wet": wet, "wqt": wqt, "qt": qt, "vt": vt,
            "maskadd": maskadd,
        })
    return in_maps


def run(query_state, encoder_hidden_states, attention_mask, W_c, V,
        trace=False, **trace_kwargs):
    from concourse.bass_utils import run_bass_kernel_spmd
    if "nc" not in _CACHE:
        _CACHE["nc"] = build_kernel_nc()
    nc = _CACHE["nc"]
    in_maps = shard_inputs(query_state, encoder_hidden_states,
                           attention_mask, W_c, V)
    res = run_bass_kernel_spmd(nc, in_maps, list(range(N_CORES)),
                               trace=trace, **trace_kwargs)
    ctx = np.concatenate([r["ctx"] for r in res.results], axis=0)
    attn = np.concatenate([r["attn"] for r in res.results], axis=0)
    return (ctx.astype(np.float32), attn.astype(np.float32)), res


def kernel(query_state, encoder_hidden_states, attention_mask, W_c, V):
    (ctx, attn), _ = run(query_state, encoder_hidden_states, attention_mask,
                         W_c, V)
    return ctx, attn


# revision 9
# speedup vs baseline: 1.0385x; 1.0385x over previous
"""Luong concat attention, data-parallel over batch across 8 TRN2 NeuronCores.

Reference computation (per batch row b):
    q_proj  = q[b] @ Wq.T                    # [DA]
    e_proj  = e[b] @ We.T                    # [S, DA]
    energies = tanh(q_proj + e_proj)         # [S, DA]
    scores  = energies @ V.T  (+ mask)       # [S]
    probs   = softmax(scores)                # [S]
    ctx     = probs @ e[b]                   # [DE]

Sharding: B=32 split 4 per core; no collectives. The dominant matmul
(e_proj, 137 GFLOP total) runs in bf16 with fp32 PSUM accumulation.

Per-core layout strategy:
  - e arrives in HBM as bf16 [4, S, DE] (host casts).
  - For e_proj the contraction is over DE, so e is DMA-transposed
    (hardware xbar) into SBUF as eT chunks [128(de), S].
  - e_proj computed as out[da_tile(128 part), tok(512 free)] so tanh's
    q_proj addend is a per-partition ACT bias, and scores = V . energies
    is a PE matmul contracting the partition dim.
  - softmax on a [1, S] row (mask pre-folded in as an additive -1e9 row
    via a K=1 matmul into the scores PSUM accumulation).
  - probs go to HBM fp32, and are gathered back transposed+cast to bf16
    [128(s), 16] via a SWDGE (gpsimd) casting DMA, which is the lhsT for
    the context matmul against natural-layout e rows.
"""

import numpy as np
import ml_dtypes

# Problem constants (full problem; per-core shard sizes derived).
B, S, DD, DE, DA = 32, 2048, 1024, 1024, 1024
N_CORES = 8
BL = B // N_CORES  # local batches per core
NEG = -1000000000.0

_CACHE = {}


def build_kernel_nc(BL=BL, S=S, DD=DD, DE=DE, DA=DA):
    """Build + compile the per-core Bass program. Returns nc."""
    import concourse.bacc as bacc
    import concourse.mybir as mybir
    import concourse.tile as tile

    f32 = mybir.dt.float32
    bf16 = mybir.dt.bfloat16

    KD = DD // 128  # dd chunks (q_proj contraction)
    KE = DE // 128  # de chunks (e_proj contraction)
    MA = DA // 128  # da tiles
    TK = 512        # token tile (free dim per matmul)
    NT = S // TK    # token tiles per batch
    SC = S // 128   # s chunks (context contraction)
    CW = min(512, DE)  # context output tile width
    DH = DE // CW   # context output tiles

    nc = bacc.Bacc("TRN2", target_bir_lowering=False, debug=False,
                   num_devices=N_CORES)

    # I/O (per-core shapes)
    e_t = nc.dram_tensor("e", [BL, S, DE], bf16, kind="ExternalInput")
    wet_t = nc.dram_tensor("wet", [DE, DA], bf16, kind="ExternalInput")
    wqt_t = nc.dram_tensor("wqt", [DD, DA], f32, kind="ExternalInput")
    qt_t = nc.dram_tensor("qt", [DD, BL], f32, kind="ExternalInput")
    vt_t = nc.dram_tensor("vt", [128, DA // 128], bf16, kind="ExternalInput")
    mask_t = nc.dram_tensor("maskadd", [1, BL * S], bf16, kind="ExternalInput")
    ctx_t = nc.dram_tensor("ctx", [BL, DE], f32, kind="ExternalOutput")
    attn_t = nc.dram_tensor("attn", [BL, S], f32, kind="ExternalOutput")

    e = e_t.ap()
    ctx_o = ctx_t.ap()
    attn_o = attn_t.ap()

    Tanh = mybir.ActivationFunctionType.Tanh
    Exp = mybir.ActivationFunctionType.Exp
    X = mybir.AxisListType.X

    with tile.TileContext(nc) as tc:
        from contextlib import ExitStack
        with ExitStack() as ctx:
            const_pool = ctx.enter_context(tc.tile_pool(name="const", bufs=1))

            # ---- resident constants -------------------------------------
            wet_sb = const_pool.tile([128, KE, DA], bf16, tag="wet")
            nc.sync.dma_start(
                out=wet_sb, in_=wet_t.ap().rearrange("(k p) a -> p k a", p=128))
            vt_sb = const_pool.tile([128, DA // 128], bf16, tag="vt")
            nc.sync.dma_start(out=vt_sb, in_=vt_t.ap())
            mask_sb = const_pool.tile([1, BL * S], bf16, tag="mask")
            nc.sync.dma_start(out=mask_sb, in_=mask_t.ap())
            qt_sb = const_pool.tile([128, KD, BL], f32, tag="qt")
            nc.sync.dma_start(
                out=qt_sb, in_=qt_t.ap().rearrange("(k p) b -> p k b", p=128))
            ones_sb = const_pool.tile([1, 1], bf16, tag="ones")
            nc.vector.memset(ones_sb, 1.0)
            qproj_sb = const_pool.tile([128, MA, BL], f32, tag="qproj")

            # ---- main pools ---------------------------------------------
            et_pool = ctx.enter_context(tc.tile_pool(name="et", bufs=2))
            en_pool = ctx.enter_context(tc.tile_pool(name="en", bufs=2))
            er_pool = ctx.enter_context(tc.tile_pool(name="er", bufs=3))
            sm_pool = ctx.enter_context(tc.tile_pool(name="sm", bufs=2))
            rd_pool = ctx.enter_context(tc.tile_pool(name="rd", bufs=4))
            pt_pool = ctx.enter_context(tc.tile_pool(name="pt", bufs=2))
            eps_pool = ctx.enter_context(
                tc.tile_pool(name="eps", bufs=3, space="PSUM"))
            sc_pool = ctx.enter_context(
                tc.tile_pool(name="scp", bufs=2, space="PSUM"))
            cx_pool = ctx.enter_context(
                tc.tile_pool(name="cxp", bufs=1, space="PSUM"))
            dr_pool = ctx.enter_context(
                tc.tile_pool(name="dr", bufs=2, space="DRAM"))

            # ---- phase 1: q_proj in fp32 (tiny, precision matters) ------
            with tc.tile_pool(name="qp", bufs=1) as wq_pool, \
                 tc.tile_pool(name="qpp", bufs=1, space="PSUM") as qp_psum:
                wq = []
                for k in range(KD):
                    w = wq_pool.tile([128, DA], f32, tag=f"wq{k}")
                    nc.gpsimd.dma_start(
                        out=w, in_=wqt_t.ap()[k * 128:(k + 1) * 128, :])
                    wq.append(w)
                for m in range(MA):
                    ps = qp_psum.tile([128, BL], f32, tag="qps")
                    for k in range(KD):
                        nc.tensor.matmul(
                            ps, lhsT=wq[k][:, m * 128:(m + 1) * 128],
                            rhs=qt_sb[:, k, :],
                            start=(k == 0), stop=(k == KD - 1))
                    nc.scalar.copy(qproj_sb[:, m, :], ps)

            def emit_context(pT, b):
                # context: ctx[b] = probs @ e[b]
                cps = [cx_pool.tile([1, CW], f32, tag=f"cps{h}",
                                    name=f"cps{h}_{b}")
                       for h in range(DH)]
                for c in range(SC):
                    er = er_pool.tile([128, DE], bf16, tag="er")
                    nc.sync.dma_start(
                        out=er, in_=e[b, c * 128:(c + 1) * 128, :])
                    for h in range(DH):
                        nc.tensor.matmul(
                            cps[h], lhsT=pT[:, c:c + 1],
                            rhs=er[:, h * CW:(h + 1) * CW],
                            start=(c == 0), stop=(c == SC - 1))
                for h in range(DH):
                    cs = sm_pool.tile([1, CW], f32, tag=f"ctx{h}",
                                      name=f"cs{h}_{b}")
                    nc.scalar.copy(cs, cps[h])
                    nc.sync.dma_start(
                        out=ctx_o[b:b + 1, h * CW:(h + 1) * CW], in_=cs)

            pending = None
            for b in range(BL):
                # transposed e chunks for this batch: eT[k] = [128(de), S]
                eT = []
                for k in range(KE):
                    t_ = et_pool.tile([128, S], bf16, tag=f"et{k}")
                    nc.sync.dma_start(
                        out=t_, in_=e[b, :, k * 128:(k + 1) * 128],
                        transpose=True)
                    eT.append(t_)

                scores_sb = sm_pool.tile([1, S], f32, tag="scores")
                for t in range(NT):
                    en_tiles = []
                    for m in range(MA):
                        ps = eps_pool.tile([128, TK], f32, tag="eps")
                        for k in range(KE):
                            nc.tensor.matmul(
                                ps,
                                lhsT=wet_sb[:, k, m * 128:(m + 1) * 128],
                                rhs=eT[k][:, t * TK:(t + 1) * TK],
                                start=(k == 0), stop=(k == KE - 1))
                        en = en_pool.tile([128, TK], bf16, tag=f"en{m}")
                        nc.scalar.activation(
                            en, ps, Tanh, bias=qproj_sb[:, m, b:b + 1])
                        en_tiles.append(en)
                    sps = sc_pool.tile([1, TK], f32, tag="sps")
                    # mask add as K=1 matmul starting the accumulation
                    nc.tensor.matmul(
                        sps, lhsT=ones_sb,
                        rhs=mask_sb[0:1, b * S + t * TK: b * S + (t + 1) * TK],
                        start=True, stop=False)
                    for m in range(MA):
                        nc.tensor.matmul(
                            sps, lhsT=vt_sb[:, m:m + 1], rhs=en_tiles[m],
                            start=False, stop=(m == MA - 1))
                    nc.scalar.copy(scores_sb[0:1, t * TK:(t + 1) * TK], sps)

                # previous batch's context MMs go here in the PE stream, so
                # this batch's softmax latency hides behind them (and the
                # next batch's e_proj hides behind this batch's softmax).
                if pending is not None:
                    emit_context(*pending)

                # softmax over [1, S]
                negmax = rd_pool.tile([1, 1], f32, tag="negmax")
                nc.vector.reduce_max(negmax, scores_sb, axis=X, negate=True)
                sumexp = rd_pool.tile([1, 1], f32, tag="sumexp")
                pexp = sm_pool.tile([1, S], f32, tag="pexp")
                nc.scalar.activation(pexp, scores_sb, Exp, bias=negmax,
                                     accum_out=sumexp)
                rec = rd_pool.tile([1, 1], f32, tag="rec")
                nc.vector.reciprocal(rec, sumexp)
                probs = sm_pool.tile([1, S], f32, tag="probs")
                nc.vector.tensor_scalar_mul(probs, pexp, rec)
                nc.sync.dma_start(out=attn_o[b:b + 1, :], in_=probs)
                pscr = dr_pool.tile([1, S], f32, tag="pscr")
                nc.sync.dma_start(out=pscr, in_=probs)

                # transposed + bf16-cast probs: [128(s), SC]
                pT = pt_pool.tile([128, SC], bf16, tag="pt")
                nc.gpsimd.dma_start(
                    out=pT, in_=pscr.rearrange("a (c p) -> p (a c)", p=128))
                pending = (pT, b)

            emit_context(*pending)

    nc.compile()
    return nc


def shard_inputs(query_state, encoder_hidden_states, attention_mask, W_c, V,
                 n_cores=N_CORES):
    """Host-side prep: slice per-core shards + layout/dtype transforms."""
    bf16 = ml_dtypes.bfloat16
    bl = query_state.shape[0] // n_cores
    dd = query_state.shape[1]
    wet = np.ascontiguousarray(W_c[:, dd:].T).astype(bf16)      # [DE, DA]
    wqt = np.ascontiguousarray(W_c[:, :dd].T).astype(np.float32)  # [DD, DA]
    vt = np.ascontiguousarray(V[0].reshape(-1, 128).T).astype(bf16)  # [128, DA/128]
    in_maps = []
    for c in range(n_cores):
        sl = slice(c * bl, (c + 1) * bl)
        e_bf = encoder_hidden_states[sl].astype(bf16)
        qt = np.ascontiguousarray(query_state[sl].T).astype(np.float32)
        maskadd = np.where(attention_mask[sl] == 0, np.float32(NEG),
                           np.float32(0.0)).astype(bf16).reshape(1, -1)
        in_maps.append({
            "e": e_bf, "wet": wet, "wqt": wqt, "qt": qt, "vt": vt,
            "maskadd": maskadd,
        })
    return in_maps


def run(query_state, encoder_hidden_states, attention_mask, W_c, V,
        trace=False, **trace_kwargs):
    from concourse.bass_utils import run_bass_kernel_spmd
    if "nc" not in _CACHE:
        _CACHE["nc"] = build_kernel_nc()
    nc = _CACHE["nc"]
    in_maps = shard_inputs(query_state, encoder_hidden_states,
                           attention_mask, W_c, V)
    res = run_bass_kernel_spmd(nc, in_maps, list(range(N_CORES)),
                               trace=trace, **trace_kwargs)
    ctx = np.concatenate([r["ctx"] for r in res.results], axis=0)
    attn = np.concatenate([r["attn"] for r in res.results], axis=0)
    return (ctx.astype(np.float32), attn.astype(np.float32)), res


def kernel(query_state, encoder_hidden_states, attention_mask, W_c, V):
    (ctx, attn), _ = run(query_state, encoder_hidden_states, attention_mask,
                         W_c, V)
    return ctx, attn
